# revision 23
# baseline (speedup 1.0000x reference)
"""Trainium2 Bass kernel for nn_Classifier (attribute-sharded MLP heads).

Reference computation (B=64, C=1280, H=W=7, A=40):
    p   = h_swish(mean(x, axis=(2,3)))            # [B, C]
    h   = h_swish(einsum("bc,acd->bad", p, W1) + b1)
    out = sigmoid(einsum("bac,ac->ba", h, W2) + b2)  # [B, A]

Sharding: 8 cores, each owns A/8 = 5 attribute heads; x replicated.

Design (evolved over NTFF trace rounds):
- All large operands ride fp8 E3M4 with power-of-2 scale folding:
    pT = 24.5*p ; W1' = W1*256/49 (=> psum z' = 128 z) ; b1' = 128 b1 ;
    W2' = 64 W2 ; descale via activation scale + stt scalar at evac.
- Host pre-transposes W1 into ONE k-major stream [P, KC, 5888] covering
  all five heads' (n0,n1) plus the 3 wide heads' n2, so every weight
  DMA is a contiguous 2D copy (HWDGE descriptor-gen was the v1 killer),
  and PSUM accumulation for ALL heads proceeds k-major, paced by the
  pooling pipeline instead of serialized after it. Only the last pair's
  256-wide n2 section trails the stream (its 20 matmuls + evac are the
  only post-stream work).
- Pooling splits across engines: gpsimd + DVE pairwise-add the two
  25-halves (s zero-padded to 50), DVE reduces the halved tensor.
- PSUM: 8 banks exactly: solo(3) + pairA(3) + pairB n0,n1(2);
  pairB n2 reuses solo's n2 bank after its evacuation.
- Evacuation stts split DVE/gpsimd; single transposed output store
  (v2 lost ~7us to 4-byte-per-partition store drain).
"""

import sys

for _p in ("/opt/trn_rl_repo",):
    if _p not in sys.path:
        sys.path.insert(0, _p)

from contextlib import ExitStack

import numpy as np
import ml_dtypes

import concourse.bass as bass
import concourse.tile as tile
from concourse import bacc, mybir

# Problem constants (hardcoded per contract)
B = 64          # batch
C = 1280        # channels / features
S = 49          # spatial H*W
SP50 = 50       # S zero-padded so the pairwise add halves evenly
A = 40          # total attribute heads
NCORES = 8
AH = A // NCORES  # heads per core = 5
P = 128
KC = C // P       # 10 contraction chunks
NS = [(0, 512), (512, 512), (1024, 256)]  # psum n-chunks of C=1280

BF = mybir.dt.bfloat16
F8 = mybir.dt.float8e3
F32 = mybir.dt.float32
AF = mybir.ActivationFunctionType
ALU = mybir.AluOpType

# scale folding constants
SC_W1 = 256.0 / 49.0
SC_B1 = 128.0
SC_W2 = 64.0
T1_SCALE = 1.0 / (6.0 * 49.0 * 2.0)
T1H_SCALE = 1.0 / (6.0 * 128.0)
SCR_SCALE = 1.0 / 8192.0

# k-major weight stream column offsets (fp8 elems per partition per k)
# [solo n0..n2 | A0 | A1 | B0n0 B1n0 | B0n1 B1n1] = 3*1280 + 2*512 + 2*512
WK_SOLO = 0
WK_A = (1280, 2560)
WK_B0 = (3840, 4352)   # heads (2,3) n0
WK_B1 = (4864, 5376)   # heads (2,3) n1
WK_COLS = 5888

# pooling pairwise-add column split: gpsimd measured ~71 G elem/s vs DVE
# ~116; 19/6 balances gp_tt against dve_tt+reduce+stt
GP_TT = 19

GORDER = ((0, 1), (2, 3), (4,))  # column-block -> heads (out layout)

_NC_CACHE = {}


def build_nc():
    nc = bacc.Bacc("TRN2", target_bir_lowering=False, name="attr_mlp")

    xT = nc.dram_tensor("xT", [C, B * SP50], F8, kind="ExternalInput")
    wk = nc.dram_tensor("wk", [P, KC, WK_COLS], F8, kind="ExternalInput")
    wb2 = nc.dram_tensor("wb2", [P, KC, 2, 256], F8, kind="ExternalInput")
    b1 = nc.dram_tensor("b1", [AH * C], BF, kind="ExternalInput")
    w2c = nc.dram_tensor("w2c", [P, 3, C], F8, kind="ExternalInput")
    b2c = nc.dram_tensor("b2c", [P, 3], F32, kind="ExternalInput")
    out = nc.dram_tensor("out", [6 * B], F32, kind="ExternalOutput")

    g_solo, g_a, g_b = (4,), (0, 1), (2, 3)
    CB = {g_a: 0, g_b: 1, g_solo: 2}

    with tile.TileContext(nc) as tc, ExitStack() as ctx:
        const = ctx.enter_context(tc.tile_pool(name="const", bufs=1))
        st = ctx.enter_context(tc.tile_pool(name="st", bufs=1))
        xp = ctx.enter_context(tc.tile_pool(name="xp", bufs=5))
        wg = ctx.enter_context(tc.tile_pool(name="wg", bufs=1))
        sp = ctx.enter_context(tc.tile_pool(name="sp", bufs=2))
        ep = ctx.enter_context(tc.tile_pool(name="ep", bufs=2))
        pp = ctx.enter_context(tc.tile_pool(name="pp", bufs=1, space="PSUM"))

        # --- constants ---
        ones = const.tile([1, B], BF)
        nc.gpsimd.memset(ones, 1.0)
        half = const.tile([P, 1], F32)
        nc.gpsimd.memset(half, 0.5)
        quart = const.tile([P, 1], F32)
        nc.gpsimd.memset(quart, 0.25)
        b1_sb = const.tile([1, AH * C], BF)
        nc.scalar.dma_start(b1_sb, b1[None, :])
        w2t = const.tile([P, 3, C], F8)
        nc.scalar.dma_start(w2t, w2c[:, :, :])
        b2t = const.tile([P, 3], F32)
        nc.scalar.dma_start(b2t, b2c[:, :])
        osb32 = const.tile([P, 96], F32)
        nc.gpsimd.memset(osb32, 0.0)

        # --- input DMAs, issued up front in consumption order ---
        xts = []
        for kp in range(KC // 2):
            xt = xp.tile([P, 2, B * SP50], F8, tag="xt")
            src = xT[kp * 2 * P:(kp + 1) * 2 * P, :].rearrange(
                "(two p) f -> p two f", p=P)
            nc.sync.dma_start(xt, src)
            xts.append(xt)
        wkt = wg.tile([P, KC, WK_COLS], F8)
        for kp in range(KC // 2):
            nc.sync.dma_start(wkt[:, kp * 2:(kp + 1) * 2],
                              wk[:, kp * 2:(kp + 1) * 2])
        wbt2 = wg.tile([P, KC, 2, 256], F8, tag="wb2", name="wb2")
        KH = 5
        for kh in range(KC // KH):
            nc.sync.dma_start(wbt2[:, kh * KH:(kh + 1) * KH],
                              wb2[:, kh * KH:(kh + 1) * KH])

        # --- stage 1: pooling + h_swish gate -> pT (fp8, 24.5*p) ---
        pT = st.tile([P, KC, B], F8, tag="pT")
        for k in range(KC):
            xsl = xts[k // 2][:, k % 2, :].rearrange(
                "p (b s) -> p b s", s=SP50)
            th = sp.tile([P, B, 25], BF, tag="th")
            nc.gpsimd.tensor_tensor(
                th[:, :, :GP_TT], xsl[:, :, 0:GP_TT],
                xsl[:, :, 25:25 + GP_TT], ALU.add)
            nc.vector.tensor_tensor(
                th[:, :, GP_TT:], xsl[:, :, GP_TT:25],
                xsl[:, :, 25 + GP_TT:50], ALU.add)
            sums = sp.tile([P, B], F32, tag="sums")
            nc.vector.reduce_sum(sums, th, axis=mybir.AxisListType.X)
            t1 = sp.tile([P, B], F32, tag="t1")
            nc.scalar.activation(t1, sums, AF.Relu, bias=quart, scale=T1_SCALE)
            nc.vector.scalar_tensor_tensor(
                pT[:, k, :], t1, 0.5, sums, ALU.min, ALU.mult)

        # --- PSUM: 8 banks exactly ---
        def psum_tiles(base, sizes):
            return [pp.tile([P, nn], F32, tag=f"{base}{ni}", name=f"{base}{ni}")
                    for ni, nn in enumerate(sizes)]

        ps_solo = psum_tiles("pss", (512, 512, 256))
        ps_a = psum_tiles("psa", (512, 512, 256))
        ps_b01 = psum_tiles("psb", (512, 512))

        def bias_mm(ps, j, a, n0, nn):
            tp = (0, 64 * j) if j else None
            nc.tensor.matmul(
                ps[64 * j:64 * j + B, :], ones,
                b1_sb[:, a * C + n0:a * C + n0 + nn],
                start=False, stop=False, tile_position=tp,
                skip_group_check=True)

        def mm(ps, j, wsl, k):
            tp = (0, 64 * j) if j else None
            nc.tensor.matmul(
                ps[64 * j:64 * j + B, :], pT[:, k, :], wsl,
                start=(k == 0), stop=(k == KC - 1),
                tile_position=tp, skip_group_check=True)

        # --- GEMM: all heads k-major (n2 of pair B deferred) ---
        for k in range(KC):
            for ni, (n0, nn) in enumerate(NS):
                mm(ps_solo[ni], 0, wkt[:, k, WK_SOLO + n0:WK_SOLO + n0 + nn], k)
            for j in (0, 1):
                for ni, (n0, nn) in enumerate(NS):
                    mm(ps_a[ni], j,
                       wkt[:, k, WK_A[j] + n0:WK_A[j] + n0 + nn], k)
            for j in (0, 1):
                mm(ps_b01[0], j, wkt[:, k, WK_B0[j]:WK_B0[j] + 512], k)
                mm(ps_b01[1], j, wkt[:, k, WK_B1[j]:WK_B1[j] + 512], k)
            if k == 0:
                for ni, (n0, nn) in enumerate(NS):
                    bias_mm(ps_solo[ni], 0, 4, n0, nn)
                    for j, a in enumerate(g_a):
                        bias_mm(ps_a[ni], j, a, n0, nn)
                for ni in (0, 1):
                    n0, nn = NS[ni]
                    for j, a in enumerate(g_b):
                        bias_mm(ps_b01[ni], j, a, n0, nn)

        # --- evacuation (t2w on gpsimd for the pairs: DVE is the tail
        # bottleneck; gpsimd is idle after pooling) ---
        def evac_chunk(hs, zs_full, ni, n0, nn, rpart, tteng):
            pn = B * len(hs)
            zs = zs_full[:pn]
            t1h = ep.tile([P, 512], F32, tag="t1h")
            nc.scalar.activation(
                t1h[:pn, :nn], zs, AF.Relu, bias=half[:pn], scale=T1H_SCALE)
            t2w = ep.tile([P, 512], BF, tag="t2w")
            tteng.scalar_tensor_tensor(
                t2w[:pn, :nn], t1h[:pn, :nn], 1.0,
                w2t[:pn, CB[hs], n0:n0 + nn], ALU.min, ALU.mult)
            scr = ep.tile([P, 512], F32, tag="scr")
            nc.vector.scalar_tensor_tensor(
                scr[:pn, :nn], zs, SCR_SCALE, t2w[:pn, :nn],
                ALU.mult, ALU.mult, accum_out=rpart[:pn, ni:ni + 1])

        def evac_fin(hs, rpart):
            pn = B * len(hs)
            cb = CB[hs]
            rlog = st.tile([P, 1], F32, tag=f"rl{hs[0]}")
            nc.vector.reduce_sum(rlog[:pn], rpart[:pn, :],
                                 axis=mybir.AxisListType.X)
            nc.scalar.activation(osb32[:pn, 32 * cb:32 * cb + 1], rlog[:pn],
                                 AF.Sigmoid, bias=b2t[:pn, cb:cb + 1])

        rp_s = st.tile([P, 3], F32, tag="rps")
        rp_a = st.tile([P, 3], F32, tag="rpa")
        rp_b = st.tile([P, 3], F32, tag="rpb")
        for ni, (n0, nn) in enumerate(NS):
            evac_chunk(g_solo, ps_solo[ni], ni, n0, nn, rp_s, nc.vector)
        evac_fin(g_solo, rp_s)
        for ni in (0, 1):
            n0, nn = NS[ni]
            evac_chunk(g_b, ps_b01[ni], ni, n0, nn, rp_b, nc.vector)
        for ni, (n0, nn) in enumerate(NS):
            evac_chunk(g_a, ps_a[ni], ni, n0, nn, rp_a, nc.vector)
        evac_fin(g_a, rp_a)

        # --- pair B n2: trailing weights; psum reuses solo's n2 bank ---
        ps_b2 = pp.tile([P, 256], F32, tag="pss2", name="pss2_b")
        n0, nn = NS[2]
        for j in (0, 1):
            for k in range(KC):
                mm(ps_b2, j, wbt2[:, k, j, :], k)
                if k == 0:
                    bias_mm(ps_b2, j, g_b[j], n0, nn)
        evac_chunk(g_b, ps_b2, 2, n0, nn, rp_b, nc.vector)
        evac_fin(g_b, rp_b)

        # --- single contiguous output store via block transpose ---
        tr = st.tile([P, 96], F32, tag="tr")
        nc.vector.transpose(tr, osb32)
        src = tr[0:P:32, :].rearrange("q (cb j) -> q cb j", j=32)
        dst = out[:].rearrange("(cb q j) -> q cb j", q=4, j=32)
        nc.sync.dma_start(dst, src)

    nc.compile()
    return nc


def get_nc():
    if "nc" not in _NC_CACHE:
        _NC_CACHE["nc"] = build_nc()
    return _NC_CACHE["nc"]


def make_in_maps(x, W1, b1, W2, b2):
    f8 = ml_dtypes.float8_e3m4
    bf = ml_dtypes.bfloat16
    x = np.asarray(x, dtype=np.float32)
    W1 = np.asarray(W1, dtype=np.float32)
    b1 = np.asarray(b1, dtype=np.float32)
    W2 = np.asarray(W2, dtype=np.float32)
    b2 = np.asarray(b2, dtype=np.float32)

    # [B, C, H, W] -> [C, B*50] (s zero-padded), replicated to all cores
    xp50 = np.zeros((C, B, SP50), np.float32)
    xp50[:, :, :S] = x.reshape(B, C, S).transpose(1, 0, 2)
    xT = xp50.reshape(C, B * SP50).astype(f8)

    in_maps = []
    for core in range(NCORES):
        a0 = core * AH
        # wh[a][k, p, d] = W1[a0+a][k*128+p, d] * SC_W1
        whs = (W1[a0:a0 + AH] * SC_W1).reshape(AH, KC, P, C)
        wt = whs.transpose(0, 2, 1, 3)  # [AH, P, KC, C]
        wkm = np.empty((P, KC, WK_COLS), np.float32)
        wkm[:, :, WK_SOLO:WK_SOLO + C] = wt[4]
        for j, a in enumerate((0, 1)):
            wkm[:, :, WK_A[j]:WK_A[j] + C] = wt[a]
        for j, a in enumerate((2, 3)):
            wkm[:, :, WK_B0[j]:WK_B0[j] + 512] = wt[a][:, :, 0:512]
            wkm[:, :, WK_B1[j]:WK_B1[j] + 512] = wt[a][:, :, 512:1024]
        wb2m = np.empty((P, KC, 2, 256), np.float32)
        for j, a in enumerate((2, 3)):
            wb2m[:, :, j, :] = wt[a][:, :, 1024:1280]

        w2s = W2[a0:a0 + AH] * SC_W2  # [AH, C]
        w2cm = np.zeros((P, 3, C), np.float32)
        b2cm = np.zeros((P, 3), np.float32)
        for cb, hs in enumerate(GORDER):
            for j, a in enumerate(hs):
                w2cm[64 * j:64 * (j + 1), cb, :] = w2s[a]
                b2cm[64 * j:64 * (j + 1), cb] = b2[a0 + a]
        in_maps.append({
            "xT": xT,
            "wk": np.ascontiguousarray(wkm).astype(f8),
            "wb2": np.ascontiguousarray(wb2m).astype(f8),
            "b1": np.ascontiguousarray(b1[a0:a0 + AH] * SC_B1
                                       ).reshape(AH * C).astype(bf),
            "w2c": w2cm.astype(f8),
            "b2c": b2cm,
        })
    return in_maps


def kernel(x, W1, b1, W2, b2, _trace=False, _tmpdir=None):
    from concourse.bass_utils import run_bass_kernel_spmd

    nc = get_nc()
    in_maps = make_in_maps(x, W1, b1, W2, b2)
    res = run_bass_kernel_spmd(
        nc, in_maps, core_ids=list(range(NCORES)),
        trace=_trace, tmpdir=_tmpdir,
    )
    # out flat is (cb, head_within, batch); reassemble to [B, AH] per core
    outs = []
    for c in range(NCORES):
        o = np.asarray(res.results[c]["out"], dtype=np.float32).reshape(6, B)
        oc = np.empty((B, AH), np.float32)
        for cb, hs in enumerate(GORDER):
            for j, a in enumerate(hs):
                oc[:, a] = o[2 * cb + j]
        outs.append(oc)
    full = np.concatenate(outs, axis=1)  # [B, A]
    if _trace:
        return full, res
    return full


# revision 24
# speedup vs baseline: 1.0036x; 1.0036x over previous
"""Trainium2 Bass kernel for nn_Classifier (attribute-sharded MLP heads).

Reference computation (B=64, C=1280, H=W=7, A=40):
    p   = h_swish(mean(x, axis=(2,3)))            # [B, C]
    h   = h_swish(einsum("bc,acd->bad", p, W1) + b1)
    out = sigmoid(einsum("bac,ac->ba", h, W2) + b2)  # [B, A]

Sharding: 8 cores, each owns A/8 = 5 attribute heads; x replicated.

Design (evolved over NTFF trace rounds):
- All large operands ride fp8 E3M4 with power-of-2 scale folding:
    pT = 24.5*p ; W1' = W1*256/49 (=> psum z' = 128 z) ; b1' = 128 b1 ;
    W2' = 64 W2 ; descale via activation scale + stt scalar at evac.
- Host pre-transposes W1 into ONE k-major stream [P, KC, 5888] covering
  all five heads' (n0,n1) plus the 3 wide heads' n2, so every weight
  DMA is a contiguous 2D copy (HWDGE descriptor-gen was the v1 killer),
  and PSUM accumulation for ALL heads proceeds k-major, paced by the
  pooling pipeline instead of serialized after it. Only the last pair's
  256-wide n2 section trails the stream (its 20 matmuls + evac are the
  only post-stream work).
- Pooling splits across engines: gpsimd + DVE pairwise-add the two
  25-halves (s zero-padded to 50), DVE reduces the halved tensor.
- PSUM: 8 banks exactly: solo(3) + pairA(3) + pairB n0,n1(2);
  pairB n2 reuses solo's n2 bank after its evacuation.
- Evacuation stts split DVE/gpsimd; single transposed output store
  (v2 lost ~7us to 4-byte-per-partition store drain).
"""

import sys

for _p in ("/opt/trn_rl_repo",):
    if _p not in sys.path:
        sys.path.insert(0, _p)

from contextlib import ExitStack

import numpy as np
import ml_dtypes

import concourse.bass as bass
import concourse.tile as tile
from concourse import bacc, mybir

# Problem constants (hardcoded per contract)
B = 64          # batch
C = 1280        # channels / features
S = 49          # spatial H*W
SP50 = 50       # S zero-padded so the pairwise add halves evenly
A = 40          # total attribute heads
NCORES = 8
AH = A // NCORES  # heads per core = 5
P = 128
KC = C // P       # 10 contraction chunks
NS = [(0, 512), (512, 512), (1024, 256)]  # psum n-chunks of C=1280

BF = mybir.dt.bfloat16
F8 = mybir.dt.float8e3
F32 = mybir.dt.float32
AF = mybir.ActivationFunctionType
ALU = mybir.AluOpType

# scale folding constants
SC_W1 = 256.0 / 49.0
SC_B1 = 128.0
SC_W2 = 64.0
T1_SCALE = 1.0 / (6.0 * 49.0 * 2.0)
T1H_SCALE = 1.0 / (6.0 * 128.0)
SCR_SCALE = 1.0 / 8192.0

# k-major weight stream column offsets (fp8 elems per partition per k)
# [solo n0..n2 | A0 | A1 | B0n0 B1n0 | B0n1 B1n1] = 3*1280 + 2*512 + 2*512
WK_SOLO = 0
WK_A = (1280, 2560)
WK_B0 = (3840, 4352)   # heads (2,3) n0
WK_B1 = (4864, 5376)   # heads (2,3) n1
WK_COLS = 5888

# pooling pairwise-add column split: gpsimd measured ~71 G elem/s vs DVE
# ~116; 19/6 balances gp_tt against dve_tt+reduce+stt
GP_TT = 19

GORDER = ((0, 1), (2, 3), (4,))  # column-block -> heads (out layout)

_NC_CACHE = {}


def build_nc():
    nc = bacc.Bacc("TRN2", target_bir_lowering=False, name="attr_mlp")

    xT = nc.dram_tensor("xT", [C, B * SP50], F8, kind="ExternalInput")
    wk = nc.dram_tensor("wk", [P, KC, WK_COLS], F8, kind="ExternalInput")
    wb2 = nc.dram_tensor("wb2", [P, KC, 2, 256], F8, kind="ExternalInput")
    b1 = nc.dram_tensor("b1", [AH * C], BF, kind="ExternalInput")
    w2c = nc.dram_tensor("w2c", [P, 3, C], F8, kind="ExternalInput")
    b2c = nc.dram_tensor("b2c", [P, 3], F32, kind="ExternalInput")
    out = nc.dram_tensor("out", [6 * B], F32, kind="ExternalOutput")

    g_solo, g_a, g_b = (4,), (0, 1), (2, 3)
    CB = {g_a: 0, g_b: 1, g_solo: 2}

    with tile.TileContext(nc) as tc, ExitStack() as ctx:
        const = ctx.enter_context(tc.tile_pool(name="const", bufs=1))
        st = ctx.enter_context(tc.tile_pool(name="st", bufs=1))
        xp = ctx.enter_context(tc.tile_pool(name="xp", bufs=5))
        wg = ctx.enter_context(tc.tile_pool(name="wg", bufs=1))
        sp = ctx.enter_context(tc.tile_pool(name="sp", bufs=3))
        ep = ctx.enter_context(tc.tile_pool(name="ep", bufs=2))
        pp = ctx.enter_context(tc.tile_pool(name="pp", bufs=1, space="PSUM"))

        # --- constants ---
        ones = const.tile([1, B], BF)
        nc.gpsimd.memset(ones, 1.0)
        half = const.tile([P, 1], F32)
        nc.gpsimd.memset(half, 0.5)
        quart = const.tile([P, 1], F32)
        nc.gpsimd.memset(quart, 0.25)
        b1_sb = const.tile([1, AH * C], BF)
        nc.scalar.dma_start(b1_sb, b1[None, :])
        w2t = const.tile([P, 3, C], F8)
        nc.scalar.dma_start(w2t, w2c[:, :, :])
        b2t = const.tile([P, 3], F32)
        nc.scalar.dma_start(b2t, b2c[:, :])
        osb32 = const.tile([P, 96], F32)
        nc.gpsimd.memset(osb32, 0.0)

        # --- input DMAs, issued up front in consumption order ---
        xts = []
        for kp in range(KC // 2):
            xt = xp.tile([P, 2, B * SP50], F8, tag="xt")
            src = xT[kp * 2 * P:(kp + 1) * 2 * P, :].rearrange(
                "(two p) f -> p two f", p=P)
            nc.sync.dma_start(xt, src)
            xts.append(xt)
        wkt = wg.tile([P, KC, WK_COLS], F8)
        for kp in range(KC // 2):
            nc.sync.dma_start(wkt[:, kp * 2:(kp + 1) * 2],
                              wk[:, kp * 2:(kp + 1) * 2])
        wbt2 = wg.tile([P, KC, 2, 256], F8, tag="wb2", name="wb2")
        KH = 5
        for kh in range(KC // KH):
            nc.sync.dma_start(wbt2[:, kh * KH:(kh + 1) * KH],
                              wb2[:, kh * KH:(kh + 1) * KH])

        # --- stage 1: pooling + h_swish gate -> pT (fp8, 24.5*p) ---
        # bf16: a DVE stt with fp8 output runs a slow path (~2.2us vs
        # ~0.2us) under concurrent engine load; bf16 stationary x fp8
        # moving matmul is legal (only fp32 must match on both sides)
        pT = st.tile([P, KC, B], BF, tag="pT")
        for k in range(KC):
            xsl = xts[k // 2][:, k % 2, :].rearrange(
                "p (b s) -> p b s", s=SP50)
            th = sp.tile([P, B, 25], BF, tag="th")
            nc.gpsimd.tensor_tensor(
                th[:, :, :GP_TT], xsl[:, :, 0:GP_TT],
                xsl[:, :, 25:25 + GP_TT], ALU.add)
            nc.vector.tensor_tensor(
                th[:, :, GP_TT:], xsl[:, :, GP_TT:25],
                xsl[:, :, 25 + GP_TT:50], ALU.add)
            sums = sp.tile([P, B], F32, tag="sums")
            nc.vector.reduce_sum(sums, th, axis=mybir.AxisListType.X)
            t1 = sp.tile([P, B], F32, tag="t1")
            nc.scalar.activation(t1, sums, AF.Relu, bias=quart, scale=T1_SCALE)
            nc.vector.scalar_tensor_tensor(
                pT[:, k, :], t1, 0.5, sums, ALU.min, ALU.mult)

        # --- PSUM: 8 banks exactly ---
        def psum_tiles(base, sizes):
            return [pp.tile([P, nn], F32, tag=f"{base}{ni}", name=f"{base}{ni}")
                    for ni, nn in enumerate(sizes)]

        ps_solo = psum_tiles("pss", (512, 512, 256))
        ps_a = psum_tiles("psa", (512, 512, 256))
        ps_b01 = psum_tiles("psb", (512, 512))

        def bias_mm(ps, j, a, n0, nn):
            tp = (0, 64 * j) if j else None
            nc.tensor.matmul(
                ps[64 * j:64 * j + B, :], ones,
                b1_sb[:, a * C + n0:a * C + n0 + nn],
                start=False, stop=False, tile_position=tp,
                skip_group_check=True)

        def mm(ps, j, wsl, k):
            tp = (0, 64 * j) if j else None
            nc.tensor.matmul(
                ps[64 * j:64 * j + B, :], pT[:, k, :], wsl,
                start=(k == 0), stop=(k == KC - 1),
                tile_position=tp, skip_group_check=True)

        # --- GEMM: all heads k-major (n2 of pair B deferred) ---
        for k in range(KC):
            for ni, (n0, nn) in enumerate(NS):
                mm(ps_solo[ni], 0, wkt[:, k, WK_SOLO + n0:WK_SOLO + n0 + nn], k)
            for j in (0, 1):
                for ni, (n0, nn) in enumerate(NS):
                    mm(ps_a[ni], j,
                       wkt[:, k, WK_A[j] + n0:WK_A[j] + n0 + nn], k)
            for j in (0, 1):
                mm(ps_b01[0], j, wkt[:, k, WK_B0[j]:WK_B0[j] + 512], k)
                mm(ps_b01[1], j, wkt[:, k, WK_B1[j]:WK_B1[j] + 512], k)
            if k == 0:
                for ni, (n0, nn) in enumerate(NS):
                    bias_mm(ps_solo[ni], 0, 4, n0, nn)
                    for j, a in enumerate(g_a):
                        bias_mm(ps_a[ni], j, a, n0, nn)
                for ni in (0, 1):
                    n0, nn = NS[ni]
                    for j, a in enumerate(g_b):
                        bias_mm(ps_b01[ni], j, a, n0, nn)

        # --- evacuation (t2w on gpsimd for the pairs: DVE is the tail
        # bottleneck; gpsimd is idle after pooling) ---
        def evac_chunk(hs, zs_full, ni, n0, nn, rpart, tteng):
            pn = B * len(hs)
            zs = zs_full[:pn]
            t1h = ep.tile([P, 512], F32, tag="t1h")
            nc.scalar.activation(
                t1h[:pn, :nn], zs, AF.Relu, bias=half[:pn], scale=T1H_SCALE)
            t2w = ep.tile([P, 512], BF, tag="t2w")
            tteng.scalar_tensor_tensor(
                t2w[:pn, :nn], t1h[:pn, :nn], 1.0,
                w2t[:pn, CB[hs], n0:n0 + nn], ALU.min, ALU.mult)
            scr = ep.tile([P, 512], F32, tag="scr")
            nc.vector.scalar_tensor_tensor(
                scr[:pn, :nn], zs, SCR_SCALE, t2w[:pn, :nn],
                ALU.mult, ALU.mult, accum_out=rpart[:pn, ni:ni + 1])

        def evac_fin(hs, rpart):
            pn = B * len(hs)
            cb = CB[hs]
            rlog = st.tile([P, 1], F32, tag=f"rl{hs[0]}")
            nc.vector.reduce_sum(rlog[:pn], rpart[:pn, :],
                                 axis=mybir.AxisListType.X)
            nc.scalar.activation(osb32[:pn, 32 * cb:32 * cb + 1], rlog[:pn],
                                 AF.Sigmoid, bias=b2t[:pn, cb:cb + 1])

        rp_s = st.tile([P, 3], F32, tag="rps")
        rp_a = st.tile([P, 3], F32, tag="rpa")
        rp_b = st.tile([P, 3], F32, tag="rpb")
        for ni, (n0, nn) in enumerate(NS):
            evac_chunk(g_solo, ps_solo[ni], ni, n0, nn, rp_s, nc.vector)
        evac_fin(g_solo, rp_s)
        for ni in (0, 1):
            n0, nn = NS[ni]
            evac_chunk(g_b, ps_b01[ni], ni, n0, nn, rp_b, nc.vector)
        for ni, (n0, nn) in enumerate(NS):
            evac_chunk(g_a, ps_a[ni], ni, n0, nn, rp_a, nc.vector)
        evac_fin(g_a, rp_a)

        # --- pair B n2: trailing weights; psum reuses solo's n2 bank ---
        ps_b2 = pp.tile([P, 256], F32, tag="pss2", name="pss2_b")
        n0, nn = NS[2]
        for j in (0, 1):
            for k in range(KC):
                mm(ps_b2, j, wbt2[:, k, j, :], k)
                if k == 0:
                    bias_mm(ps_b2, j, g_b[j], n0, nn)
        evac_chunk(g_b, ps_b2, 2, n0, nn, rp_b, nc.vector)
        evac_fin(g_b, rp_b)

        # --- single contiguous output store via block transpose ---
        tr = st.tile([P, 96], F32, tag="tr")
        nc.vector.transpose(tr, osb32)
        src = tr[0:P:32, :].rearrange("q (cb j) -> q cb j", j=32)
        dst = out[:].rearrange("(cb q j) -> q cb j", q=4, j=32)
        nc.sync.dma_start(dst, src)

    nc.compile()
    return nc


def get_nc():
    if "nc" not in _NC_CACHE:
        _NC_CACHE["nc"] = build_nc()
    return _NC_CACHE["nc"]


def make_in_maps(x, W1, b1, W2, b2):
    f8 = ml_dtypes.float8_e3m4
    bf = ml_dtypes.bfloat16
    x = np.asarray(x, dtype=np.float32)
    W1 = np.asarray(W1, dtype=np.float32)
    b1 = np.asarray(b1, dtype=np.float32)
    W2 = np.asarray(W2, dtype=np.float32)
    b2 = np.asarray(b2, dtype=np.float32)

    # [B, C, H, W] -> [C, B*50] (s zero-padded), replicated to all cores
    xp50 = np.zeros((C, B, SP50), np.float32)
    xp50[:, :, :S] = x.reshape(B, C, S).transpose(1, 0, 2)
    xT = xp50.reshape(C, B * SP50).astype(f8)

    in_maps = []
    for core in range(NCORES):
        a0 = core * AH
        # wh[a][k, p, d] = W1[a0+a][k*128+p, d] * SC_W1
        whs = (W1[a0:a0 + AH] * SC_W1).reshape(AH, KC, P, C)
        wt = whs.transpose(0, 2, 1, 3)  # [AH, P, KC, C]
        wkm = np.empty((P, KC, WK_COLS), np.float32)
        wkm[:, :, WK_SOLO:WK_SOLO + C] = wt[4]
        for j, a in enumerate((0, 1)):
            wkm[:, :, WK_A[j]:WK_A[j] + C] = wt[a]
        for j, a in enumerate((2, 3)):
            wkm[:, :, WK_B0[j]:WK_B0[j] + 512] = wt[a][:, :, 0:512]
            wkm[:, :, WK_B1[j]:WK_B1[j] + 512] = wt[a][:, :, 512:1024]
        wb2m = np.empty((P, KC, 2, 256), np.float32)
        for j, a in enumerate((2, 3)):
            wb2m[:, :, j, :] = wt[a][:, :, 1024:1280]

        w2s = W2[a0:a0 + AH] * SC_W2  # [AH, C]
        w2cm = np.zeros((P, 3, C), np.float32)
        b2cm = np.zeros((P, 3), np.float32)
        for cb, hs in enumerate(GORDER):
            for j, a in enumerate(hs):
                w2cm[64 * j:64 * (j + 1), cb, :] = w2s[a]
                b2cm[64 * j:64 * (j + 1), cb] = b2[a0 + a]
        in_maps.append({
            "xT": xT,
            "wk": np.ascontiguousarray(wkm).astype(f8),
            "wb2": np.ascontiguousarray(wb2m).astype(f8),
            "b1": np.ascontiguousarray(b1[a0:a0 + AH] * SC_B1
                                       ).reshape(AH * C).astype(bf),
            "w2c": w2cm.astype(f8),
            "b2c": b2cm,
        })
    return in_maps


def kernel(x, W1, b1, W2, b2, _trace=False, _tmpdir=None):
    from concourse.bass_utils import run_bass_kernel_spmd

    nc = get_nc()
    in_maps = make_in_maps(x, W1, b1, W2, b2)
    res = run_bass_kernel_spmd(
        nc, in_maps, core_ids=list(range(NCORES)),
        trace=_trace, tmpdir=_tmpdir,
    )
    # out flat is (cb, head_within, batch); reassemble to [B, AH] per core
    outs = []
    for c in range(NCORES):
        o = np.asarray(res.results[c]["out"], dtype=np.float32).reshape(6, B)
        oc = np.empty((B, AH), np.float32)
        for cb, hs in enumerate(GORDER):
            for j, a in enumerate(hs):
                oc[:, a] = o[2 * cb + j]
        outs.append(oc)
    full = np.concatenate(outs, axis=1)  # [B, A]
    if _trace:
        return full, res
    return full


# revision 25
# speedup vs baseline: 1.0682x; 1.0643x over previous
"""Trainium2 Bass kernel for nn_Classifier (attribute-sharded MLP heads).

Reference computation (B=64, C=1280, H=W=7, A=40):
    p   = h_swish(mean(x, axis=(2,3)))            # [B, C]
    h   = h_swish(einsum("bc,acd->bad", p, W1) + b1)
    out = sigmoid(einsum("bac,ac->ba", h, W2) + b2)  # [B, A]

Sharding: 8 cores, each owns A/8 = 5 attribute heads; x replicated.

Design (evolved over NTFF trace rounds):
- All large operands ride fp8 E3M4 with power-of-2 scale folding:
    pT = 24.5*p ; W1' = W1*256/49 (=> psum z' = 128 z) ; b1' = 128 b1 ;
    W2' = 64 W2 ; descale via activation scale + stt scalar at evac.
- Host pre-transposes W1 into ONE k-major stream [P, KC, 5888] covering
  all five heads' (n0,n1) plus the 3 wide heads' n2, so every weight
  DMA is a contiguous 2D copy (HWDGE descriptor-gen was the v1 killer),
  and PSUM accumulation for ALL heads proceeds k-major, paced by the
  pooling pipeline instead of serialized after it. Only the last pair's
  256-wide n2 section trails the stream (its 20 matmuls + evac are the
  only post-stream work).
- Pooling splits across engines: gpsimd + DVE pairwise-add the two
  25-halves (s zero-padded to 50), DVE reduces the halved tensor.
- PSUM: 8 banks exactly: solo(3) + pairA(3) + pairB n0,n1(2);
  pairB n2 reuses solo's n2 bank after its evacuation.
- Evacuation stts split DVE/gpsimd; single transposed output store
  (v2 lost ~7us to 4-byte-per-partition store drain).
"""

import sys

for _p in ("/opt/trn_rl_repo",):
    if _p not in sys.path:
        sys.path.insert(0, _p)

from contextlib import ExitStack

import numpy as np
import ml_dtypes

import concourse.bass as bass
import concourse.tile as tile
from concourse import bacc, mybir

# Problem constants (hardcoded per contract)
B = 64          # batch
C = 1280        # channels / features
S = 49          # spatial H*W
SP52 = 52       # S zero-padded: halves evenly AND halved rows are
                # 52B (4B-aligned) so the bf16 reduce can run 2x
A = 40          # total attribute heads
NCORES = 8
AH = A // NCORES  # heads per core = 5
P = 128
KC = C // P       # 10 contraction chunks
NS = [(0, 512), (512, 512), (1024, 256)]  # psum n-chunks of C=1280

BF = mybir.dt.bfloat16
F8 = mybir.dt.float8e3
F32 = mybir.dt.float32
AF = mybir.ActivationFunctionType
ALU = mybir.AluOpType

# scale folding constants
SC_W1 = 256.0 / 49.0
SC_B1 = 128.0
SC_W2 = 64.0
T1_SCALE = 1.0 / (6.0 * 49.0 * 2.0)
T1H_SCALE = 1.0 / (6.0 * 128.0)
SCR_SCALE = 1.0 / 8192.0

# k-major weight stream column offsets (fp8 elems per partition per k)
# [solo n0..n2 | A0 | A1 | B0n0 B1n0 | B0n1 B1n1] = 3*1280 + 2*512 + 2*512
WK_SOLO = 0
WK_A = (1280, 2560)
WK_B0 = (3840, 4352)   # heads (2,3) n0
WK_B1 = (4864, 5376)   # heads (2,3) n1
WK_COLS = 5888

GORDER = ((0, 1), (2, 3), (4,))  # column-block -> heads (out layout)

_NC_CACHE = {}


def build_nc():
    nc = bacc.Bacc("TRN2", target_bir_lowering=False, name="attr_mlp")

    xT = nc.dram_tensor("xT", [C, B * SP52], F8, kind="ExternalInput")
    wk = nc.dram_tensor("wk", [P, KC, WK_COLS], F8, kind="ExternalInput")
    wb2 = nc.dram_tensor("wb2", [P, KC, 2, 256], F8, kind="ExternalInput")
    b1 = nc.dram_tensor("b1", [AH * C], BF, kind="ExternalInput")
    w2c = nc.dram_tensor("w2c", [P, 3, C], F8, kind="ExternalInput")
    b2c = nc.dram_tensor("b2c", [P, 3], F32, kind="ExternalInput")
    out = nc.dram_tensor("out", [6 * B], F32, kind="ExternalOutput")

    g_solo, g_a, g_b = (4,), (0, 1), (2, 3)
    CB = {g_a: 0, g_b: 1, g_solo: 2}

    with tile.TileContext(nc) as tc, ExitStack() as ctx:
        const = ctx.enter_context(tc.tile_pool(name="const", bufs=1))
        st = ctx.enter_context(tc.tile_pool(name="st", bufs=1))
        xp = ctx.enter_context(tc.tile_pool(name="xp", bufs=5))
        wg = ctx.enter_context(tc.tile_pool(name="wg", bufs=1))
        sp = ctx.enter_context(tc.tile_pool(name="sp", bufs=3))
        ep = ctx.enter_context(tc.tile_pool(name="ep", bufs=2))
        pp = ctx.enter_context(tc.tile_pool(name="pp", bufs=1, space="PSUM"))

        # --- constants ---
        ones = const.tile([1, B], BF)
        nc.gpsimd.memset(ones, 1.0)
        half = const.tile([P, 1], F32)
        nc.gpsimd.memset(half, 0.5)
        quart = const.tile([P, 1], F32)
        nc.gpsimd.memset(quart, 0.25)
        b1_sb = const.tile([1, AH * C], BF)
        nc.scalar.dma_start(b1_sb, b1[None, :])
        w2t = const.tile([P, 3, C], F8)
        nc.scalar.dma_start(w2t, w2c[:, :, :])
        b2t = const.tile([P, 3], F32)
        nc.scalar.dma_start(b2t, b2c[:, :])
        osb32 = const.tile([P, 96], F32)
        nc.gpsimd.memset(osb32, 0.0)

        # --- input DMAs, issued up front in consumption order ---
        xts = []
        for kp in range(KC // 2):
            xt = xp.tile([P, 2, B * SP52], F8, tag="xt")
            src = xT[kp * 2 * P:(kp + 1) * 2 * P, :].rearrange(
                "(two p) f -> p two f", p=P)
            nc.sync.dma_start(xt, src)
            xts.append(xt)
        wkt = wg.tile([P, KC, WK_COLS], F8)
        for kp in range(KC // 2):
            nc.sync.dma_start(wkt[:, kp * 2:(kp + 1) * 2],
                              wk[:, kp * 2:(kp + 1) * 2])
        wbt2 = wg.tile([P, KC, 2, 256], F8, tag="wb2", name="wb2")
        KH = 5
        for kh in range(KC // KH):
            nc.sync.dma_start(wbt2[:, kh * KH:(kh + 1) * KH],
                              wb2[:, kh * KH:(kh + 1) * KH])

        # --- stage 1: pooling + h_swish gate -> pT (fp8, 24.5*p) ---
        # bf16: a DVE stt with fp8 output runs a slow path (~2.2us vs
        # ~0.2us) under concurrent engine load; bf16 stationary x fp8
        # moving matmul is legal (only fp32 must match on both sides)
        pT = st.tile([P, KC, B], BF, tag="pT")
        # all-DVE: concurrent gpsimd activity stalls DVE write-heavy
        # instructions engine-level (measured: DVE stt/TT stretch to the
        # exact end of any in-flight gpsimd TT, ~4x slowdown)
        for k in range(KC):
            xsl = xts[k // 2][:, k % 2, :].rearrange(
                "p (b s) -> p b s", s=SP52)
            th = sp.tile([P, B, 26], BF, tag="th")
            nc.vector.tensor_tensor(
                th, xsl[:, :, 0:26], xsl[:, :, 26:52], ALU.add)
            sums = sp.tile([P, B], F32, tag="sums")
            nc.vector.reduce_sum(sums, th, axis=mybir.AxisListType.X)
            t1 = sp.tile([P, B], F32, tag="t1")
            nc.scalar.activation(t1, sums, AF.Relu, bias=quart, scale=T1_SCALE)
            nc.vector.scalar_tensor_tensor(
                pT[:, k, :], t1, 0.5, sums, ALU.min, ALU.mult)

        # --- PSUM: 8 banks exactly ---
        def psum_tiles(base, sizes):
            return [pp.tile([P, nn], F32, tag=f"{base}{ni}", name=f"{base}{ni}")
                    for ni, nn in enumerate(sizes)]

        ps_solo = psum_tiles("pss", (512, 512, 256))
        ps_a = psum_tiles("psa", (512, 512, 256))
        ps_b01 = psum_tiles("psb", (512, 512))

        def bias_mm(ps, j, a, n0, nn):
            tp = (0, 64 * j) if j else None
            nc.tensor.matmul(
                ps[64 * j:64 * j + B, :], ones,
                b1_sb[:, a * C + n0:a * C + n0 + nn],
                start=False, stop=False, tile_position=tp,
                skip_group_check=True)

        def mm(ps, j, wsl, k):
            tp = (0, 64 * j) if j else None
            nc.tensor.matmul(
                ps[64 * j:64 * j + B, :], pT[:, k, :], wsl,
                start=(k == 0), stop=(k == KC - 1),
                tile_position=tp, skip_group_check=True)

        # --- GEMM: all heads k-major (n2 of pair B deferred) ---
        for k in range(KC):
            for ni, (n0, nn) in enumerate(NS):
                mm(ps_solo[ni], 0, wkt[:, k, WK_SOLO + n0:WK_SOLO + n0 + nn], k)
            for j in (0, 1):
                for ni, (n0, nn) in enumerate(NS):
                    mm(ps_a[ni], j,
                       wkt[:, k, WK_A[j] + n0:WK_A[j] + n0 + nn], k)
            for j in (0, 1):
                mm(ps_b01[0], j, wkt[:, k, WK_B0[j]:WK_B0[j] + 512], k)
                mm(ps_b01[1], j, wkt[:, k, WK_B1[j]:WK_B1[j] + 512], k)
            if k == 0:
                for ni, (n0, nn) in enumerate(NS):
                    bias_mm(ps_solo[ni], 0, 4, n0, nn)
                    for j, a in enumerate(g_a):
                        bias_mm(ps_a[ni], j, a, n0, nn)
                for ni in (0, 1):
                    n0, nn = NS[ni]
                    for j, a in enumerate(g_b):
                        bias_mm(ps_b01[ni], j, a, n0, nn)

        # --- evacuation (t2w on gpsimd for the pairs: DVE is the tail
        # bottleneck; gpsimd is idle after pooling) ---
        def evac_chunk(hs, zs_full, ni, n0, nn, rpart, tteng):
            pn = B * len(hs)
            zs = zs_full[:pn]
            t1h = ep.tile([P, 512], F32, tag="t1h")
            nc.scalar.activation(
                t1h[:pn, :nn], zs, AF.Relu, bias=half[:pn], scale=T1H_SCALE)
            t2w = ep.tile([P, 512], BF, tag="t2w")
            tteng.scalar_tensor_tensor(
                t2w[:pn, :nn], t1h[:pn, :nn], 1.0,
                w2t[:pn, CB[hs], n0:n0 + nn], ALU.min, ALU.mult)
            scr = ep.tile([P, 512], F32, tag="scr")
            nc.vector.scalar_tensor_tensor(
                scr[:pn, :nn], zs, SCR_SCALE, t2w[:pn, :nn],
                ALU.mult, ALU.mult, accum_out=rpart[:pn, ni:ni + 1])

        def evac_fin(hs, rpart):
            pn = B * len(hs)
            cb = CB[hs]
            rlog = st.tile([P, 1], F32, tag=f"rl{hs[0]}")
            nc.vector.reduce_sum(rlog[:pn], rpart[:pn, :],
                                 axis=mybir.AxisListType.X)
            nc.scalar.activation(osb32[:pn, 32 * cb:32 * cb + 1], rlog[:pn],
                                 AF.Sigmoid, bias=b2t[:pn, cb:cb + 1])

        rp_s = st.tile([P, 3], F32, tag="rps")
        rp_a = st.tile([P, 3], F32, tag="rpa")
        rp_b = st.tile([P, 3], F32, tag="rpb")
        for ni, (n0, nn) in enumerate(NS):
            evac_chunk(g_solo, ps_solo[ni], ni, n0, nn, rp_s, nc.vector)
        evac_fin(g_solo, rp_s)
        for ni in (0, 1):
            n0, nn = NS[ni]
            evac_chunk(g_b, ps_b01[ni], ni, n0, nn, rp_b, nc.vector)
        for ni, (n0, nn) in enumerate(NS):
            evac_chunk(g_a, ps_a[ni], ni, n0, nn, rp_a, nc.vector)
        evac_fin(g_a, rp_a)

        # --- pair B n2: trailing weights; psum reuses solo's n2 bank ---
        ps_b2 = pp.tile([P, 256], F32, tag="pss2", name="pss2_b")
        n0, nn = NS[2]
        for j in (0, 1):
            for k in range(KC):
                mm(ps_b2, j, wbt2[:, k, j, :], k)
                if k == 0:
                    bias_mm(ps_b2, j, g_b[j], n0, nn)
        evac_chunk(g_b, ps_b2, 2, n0, nn, rp_b, nc.vector)
        evac_fin(g_b, rp_b)

        # --- single contiguous output store via block transpose ---
        tr = st.tile([P, 96], F32, tag="tr")
        nc.vector.transpose(tr, osb32)
        src = tr[0:P:32, :].rearrange("q (cb j) -> q cb j", j=32)
        dst = out[:].rearrange("(cb q j) -> q cb j", q=4, j=32)
        nc.sync.dma_start(dst, src)

    nc.compile()
    return nc


def get_nc():
    if "nc" not in _NC_CACHE:
        _NC_CACHE["nc"] = build_nc()
    return _NC_CACHE["nc"]


def make_in_maps(x, W1, b1, W2, b2):
    f8 = ml_dtypes.float8_e3m4
    bf = ml_dtypes.bfloat16
    x = np.asarray(x, dtype=np.float32)
    W1 = np.asarray(W1, dtype=np.float32)
    b1 = np.asarray(b1, dtype=np.float32)
    W2 = np.asarray(W2, dtype=np.float32)
    b2 = np.asarray(b2, dtype=np.float32)

    # [B, C, H, W] -> [C, B*50] (s zero-padded), replicated to all cores
    xp50 = np.zeros((C, B, SP52), np.float32)
    xp50[:, :, :S] = x.reshape(B, C, S).transpose(1, 0, 2)
    xT = xp50.reshape(C, B * SP52).astype(f8)

    in_maps = []
    for core in range(NCORES):
        a0 = core * AH
        # wh[a][k, p, d] = W1[a0+a][k*128+p, d] * SC_W1
        whs = (W1[a0:a0 + AH] * SC_W1).reshape(AH, KC, P, C)
        wt = whs.transpose(0, 2, 1, 3)  # [AH, P, KC, C]
        wkm = np.empty((P, KC, WK_COLS), np.float32)
        wkm[:, :, WK_SOLO:WK_SOLO + C] = wt[4]
        for j, a in enumerate((0, 1)):
            wkm[:, :, WK_A[j]:WK_A[j] + C] = wt[a]
        for j, a in enumerate((2, 3)):
            wkm[:, :, WK_B0[j]:WK_B0[j] + 512] = wt[a][:, :, 0:512]
            wkm[:, :, WK_B1[j]:WK_B1[j] + 512] = wt[a][:, :, 512:1024]
        wb2m = np.empty((P, KC, 2, 256), np.float32)
        for j, a in enumerate((2, 3)):
            wb2m[:, :, j, :] = wt[a][:, :, 1024:1280]

        w2s = W2[a0:a0 + AH] * SC_W2  # [AH, C]
        w2cm = np.zeros((P, 3, C), np.float32)
        b2cm = np.zeros((P, 3), np.float32)
        for cb, hs in enumerate(GORDER):
            for j, a in enumerate(hs):
                w2cm[64 * j:64 * (j + 1), cb, :] = w2s[a]
                b2cm[64 * j:64 * (j + 1), cb] = b2[a0 + a]
        in_maps.append({
            "xT": xT,
            "wk": np.ascontiguousarray(wkm).astype(f8),
            "wb2": np.ascontiguousarray(wb2m).astype(f8),
            "b1": np.ascontiguousarray(b1[a0:a0 + AH] * SC_B1
                                       ).reshape(AH * C).astype(bf),
            "w2c": w2cm.astype(f8),
            "b2c": b2cm,
        })
    return in_maps


def kernel(x, W1, b1, W2, b2, _trace=False, _tmpdir=None):
    from concourse.bass_utils import run_bass_kernel_spmd

    nc = get_nc()
    in_maps = make_in_maps(x, W1, b1, W2, b2)
    res = run_bass_kernel_spmd(
        nc, in_maps, core_ids=list(range(NCORES)),
        trace=_trace, tmpdir=_tmpdir,
    )
    # out flat is (cb, head_within, batch); reassemble to [B, AH] per core
    outs = []
    for c in range(NCORES):
        o = np.asarray(res.results[c]["out"], dtype=np.float32).reshape(6, B)
        oc = np.empty((B, AH), np.float32)
        for cb, hs in enumerate(GORDER):
            for j, a in enumerate(hs):
                oc[:, a] = o[2 * cb + j]
        outs.append(oc)
    full = np.concatenate(outs, axis=1)  # [B, A]
    if _trace:
        return full, res
    return full
